# revision 1
# baseline (speedup 1.0000x reference)
"""2-layer GCN encoder on 8 Trainium2 NeuronCores (Bass/Tile).

Algorithm (per layer, using GCNConv linearity: A_hat @ (x @ W) == (A_hat @ x) @ W):
  dinv = 1/sqrt(deg+1);  htab = bf16(dinv * x_layer_input)     (node-sharded)
  AllGather htab -> full gather table in DRAM
  per dst-node tile of 128: dma_gather the htab rows of all in-edges (plus a
  self-loop edge per node), reduce into PSUM via one-hot matmuls
  (aggT[feat,dst] += msg_chunk^T @ onehot(dst_local)), then
  z = aggT^T @ W (natural layout via lhsT=aggT), post-scale by dinv, bias, relu.

Host side does only index preprocessing (edge partitioning/sorting/padding)
and sharding; all float math runs on device.
"""

import math
import numpy as np
import ml_dtypes
from contextlib import ExitStack

# ---- static problem config (hardcoded per contract) ----
N = 100000
E = 1600000
DIN = 128
DH = 128
DOUT = 64
NCORES = 8
NPC = N // NCORES            # 12500 nodes per core
NT = math.ceil(NPC / 128)    # 98 dst tiles per core
LAST_ROWS = NPC - (NT - 1) * 128   # 84
WIN = 32768                  # int16 index window for dma_gather
NW = math.ceil(N / WIN)      # 4 windows
SENTINEL = 200.0             # dst_local value for pad slots (matches no iota lane)

_CACHE = {}


def _preprocess(edge_index):
    """Partition/sort/pad edges. Returns per-core index arrays + shared schedule."""
    src = np.ascontiguousarray(edge_index[0]).astype(np.int64)
    dst = np.ascontiguousarray(edge_index[1]).astype(np.int64)

    deg = np.bincount(dst, minlength=N).astype(np.float64) + 1.0
    dinv = (1.0 / np.sqrt(deg)).astype(np.float32)

    per_core = []
    counts = np.zeros((NCORES, NT * NW), dtype=np.int64)
    for c in range(NCORES):
        lo, hi = c * NPC, (c + 1) * NPC
        sel = (dst >= lo) & (dst < hi)
        es = np.concatenate([src[sel], np.arange(lo, hi, dtype=np.int64)])
        ed = np.concatenate([dst[sel] - lo, np.arange(NPC, dtype=np.int64)])
        t = ed >> 7
        w = es // WIN
        gid = t * NW + w
        order = np.argsort(gid, kind="stable")
        es, ed, gid = es[order], ed[order], gid[order]
        counts[c] = np.bincount(gid, minlength=NT * NW)
        per_core.append((es, ed, gid))

    cnt_max = counts.max(axis=0)
    cnt_pad = ((cnt_max + 127) // 128) * 128          # 0 stays 0
    slot_off = np.zeros(NT * NW, dtype=np.int64)
    slot_off[1:] = np.cumsum(cnt_pad)[:-1]
    TOT = int(cnt_pad.sum())

    # shared gather schedule: per tile, list of (window, n_chunks, slot_off)
    sched = []
    for t in range(NT):
        ws = []
        for w in range(NW):
            g = t * NW + w
            if cnt_pad[g] > 0:
                ws.append((w, int(cnt_pad[g] // 128), int(slot_off[g])))
        sched.append(ws)

    idx_maps, dl_maps = [], []
    for c in range(NCORES):
        es, ed, gid = per_core[c]
        cstart = np.zeros(NT * NW, dtype=np.int64)
        cstart[1:] = np.cumsum(counts[c])[:-1]
        rank = np.arange(len(es)) - cstart[gid]
        slot = slot_off[gid] + rank
        idx = np.zeros(TOT, dtype=np.int16)
        dl = np.full(TOT, SENTINEL, dtype=np.float32)
        idx[slot] = (es - (es // WIN) * WIN).astype(np.int16)
        dl[slot] = (ed & 127).astype(np.float32)
        # SBUF layouts: idx wrapped over 16 partitions (replicated x8),
        # dstloc wrapped over 128 partitions, one column per 128-edge chunk.
        idx_sb = np.tile(np.ascontiguousarray(idx.reshape(-1, 16).T), (8, 1))
        dl_sb = np.ascontiguousarray(dl.reshape(-1, 128).T).astype(ml_dtypes.bfloat16)
        idx_maps.append(idx_sb)
        dl_maps.append(dl_sb)

    return dinv, idx_maps, dl_maps, sched, TOT


def _build(sched, TOT, b1_nz, b2_nz):
    import concourse.bass as bass
    import concourse.tile as tile
    from concourse import bacc, mybir

    f32 = mybir.dt.float32
    bf16 = mybir.dt.bfloat16
    AF = mybir.ActivationFunctionType
    OP = mybir.AluOpType

    nc = bacc.Bacc("TRN2", target_bir_lowering=False, debug=False,
                   num_devices=NCORES)

    x_d = nc.dram_tensor("x", [NT * 128, DIN], f32, kind="ExternalInput").ap()
    dinv_d = nc.dram_tensor("dinv", [128, NT], f32, kind="ExternalInput").ap()
    idx_d = nc.dram_tensor("idx", [128, TOT // 16], mybir.dt.int16,
                           kind="ExternalInput").ap()
    dl_d = nc.dram_tensor("dstloc", [128, TOT // 128], bf16,
                          kind="ExternalInput").ap()
    W1_d = nc.dram_tensor("W1", [DIN, DH], f32, kind="ExternalInput").ap()
    W2_d = nc.dram_tensor("W2", [DH, DOUT], f32, kind="ExternalInput").ap()
    b1_d = nc.dram_tensor("b1", [128, DH], f32, kind="ExternalInput").ap()
    b2_d = nc.dram_tensor("b2", [128, DOUT], f32, kind="ExternalInput").ap()
    out_d = nc.dram_tensor("out", [NPC, DOUT], f32, kind="ExternalOutput").ap()

    groups = [list(range(NCORES))]

    with tile.TileContext(nc) as tc, ExitStack() as ctx:
        dram = ctx.enter_context(tc.tile_pool(name="dram", bufs=1, space="DRAM"))
        tab1_shard = dram.tile([NPC, DIN], bf16)
        tab1_full = dram.tile([N, DIN], bf16, addr_space="Shared")
        tab2_shard = dram.tile([NPC, DH], bf16)
        tab2_full = dram.tile([N, DH], bf16, addr_space="Shared")

        const = ctx.enter_context(tc.tile_pool(name="const", bufs=1))
        xpool = ctx.enter_context(tc.tile_pool(name="xp", bufs=3))
        hpool = ctx.enter_context(tc.tile_pool(name="hp", bufs=3))
        msgpool = ctx.enter_context(tc.tile_pool(name="msg", bufs=3))
        mpool = ctx.enter_context(tc.tile_pool(name="mm", bufs=3))
        cppool = ctx.enter_context(tc.tile_pool(name="cp", bufs=3))
        upool = ctx.enter_context(tc.tile_pool(name="up", bufs=3))
        psA = ctx.enter_context(tc.tile_pool(name="psA", bufs=2, space="PSUM"))
        psB = ctx.enter_context(tc.tile_pool(name="psB", bufs=2, space="PSUM"))

        # ---- constants ----
        iota_i = const.tile([128, 128], mybir.dt.int32)
        nc.gpsimd.iota(iota_i[:], pattern=[[1, 128]], base=0, channel_multiplier=0)
        iota_b = const.tile([128, 128], bf16)
        nc.vector.tensor_copy(iota_b[:], iota_i[:])

        dinv_t = const.tile([128, NT], f32)
        nc.sync.dma_start(dinv_t[:], dinv_d[:])
        idx_t = const.tile([128, TOT // 16], mybir.dt.int16)
        nc.sync.dma_start(idx_t[:], idx_d[:])
        dl_t = const.tile([128, TOT // 128], bf16)
        nc.sync.dma_start(dl_t[:], dl_d[:])

        W1f = const.tile([DIN, DH], f32)
        nc.sync.dma_start(W1f[:], W1_d[:])
        W1b = const.tile([DIN, DH], bf16)
        nc.vector.tensor_copy(W1b[:], W1f[:])
        W2f = const.tile([DH, DOUT], f32)
        nc.sync.dma_start(W2f[:], W2_d[:])
        W2b = const.tile([DH, DOUT], bf16)
        nc.vector.tensor_copy(W2b[:], W2f[:])
        if b1_nz:
            b1r = const.tile([128, DH], f32)
            nc.sync.dma_start(b1r[:], b1_d[:])
        if b2_nz:
            b2r = const.tile([128, DOUT], f32)
            nc.sync.dma_start(b2r[:], b2_d[:])

        def rows_of(t):
            return LAST_ROWS if t == NT - 1 else 128

        # ---- phase 1: layer-1 gather table (h1 = dinv * x, bf16) ----
        for t in range(NT):
            xt = xpool.tile([128, DIN], f32, tag="xt")
            nc.sync.dma_start(xt[:], x_d[t * 128:(t + 1) * 128, :])
            h1 = hpool.tile([128, DIN], bf16, tag="h1")
            nc.scalar.activation(h1[:], xt[:], AF.Copy, scale=dinv_t[:, t:t + 1])
            r = rows_of(t)
            nc.sync.dma_start(tab1_shard[t * 128:t * 128 + r, :], h1[0:r, :])

        nc.gpsimd.collective_compute(
            "AllGather", OP.bypass, replica_groups=groups,
            ins=[tab1_shard[:].opt()], outs=[tab1_full[:].opt()])

        def aggregate(t, tab_full, dim):
            """Gather in-edge rows for dst tile t and reduce into PSUM.
            Returns aggT PSUM tile [dim(feat), 128(dst)]."""
            ws = sched[t]
            CH = sum(nch for (_, nch, _) in ws)
            msg = msgpool.tile([128, CH, dim], bf16, tag="msg")
            cum = 0
            for (w, nch, soff) in ws:
                wrows = min(WIN, N - w * WIN)
                nc.gpsimd.dma_gather(
                    msg[:, cum:cum + nch, :],
                    tab_full[w * WIN:w * WIN + wrows, :],
                    idx_t[:, soff // 16: soff // 16 + nch * 8],
                    num_idxs=nch * 128,
                    num_idxs_reg=nch * 128,
                    elem_size=dim,
                )
                cum += nch
            # one-hot M: [128(edge), CH*128(dst-lane)]
            cb = None
            for (w, nch, soff) in ws:
                if cb is None:
                    cb = soff // 128
            M = mpool.tile([128, CH * 128], bf16, tag="M")
            m_ap = M[:]
            out3 = bass.AP(m_ap.tensor, m_ap.offset,
                           [list(m_ap.ap[0]), [128, CH], [1, 128]])
            in0 = dl_t[:, cb:cb + CH].to_broadcast([128, CH, 128])
            io_ap = iota_b[:]
            in1 = bass.AP(io_ap.tensor, io_ap.offset,
                          [list(io_ap.ap[0]), [0, CH], [1, 128]])
            nc.vector.tensor_tensor(out3, in0, in1, op=OP.is_equal)

            agg = psA.tile([dim, 128], f32, tag="agg")
            for k in range(CH):
                nc.tensor.matmul(
                    out=agg[:],
                    lhsT=msg[:, k:k + 1, :].opt(),
                    rhs=M[:, k * 128:(k + 1) * 128],
                    start=(k == 0), stop=(k == CH - 1))
            return agg

        # ---- phase 2: layer 1 aggregate + transform -> layer-2 table ----
        for t in range(NT):
            agg = aggregate(t, tab1_full, DIN)
            cp = cppool.tile([DIN, 128], bf16, tag="cp")
            nc.scalar.activation(cp[:], agg[:], AF.Copy)
            z1 = psB.tile([128, DH], f32, tag="z1")
            nc.tensor.matmul(out=z1[:], lhsT=cp[:], rhs=W1b[:],
                             start=True, stop=True)
            # h2 = dinv * relu(dinv * z1 + b1)  (dinv per-partition here)
            if b1_nz:
                u = upool.tile([128, DH], f32, tag="u")
                nc.scalar.activation(u[:], z1[:], AF.Copy,
                                     scale=dinv_t[:, t:t + 1])
                v = upool.tile([128, DH], f32, tag="v")
                nc.vector.tensor_tensor(v[:], u[:], b1r[:], op=OP.add)
                h2 = hpool.tile([128, DH], bf16, tag="h2")
                nc.scalar.activation(h2[:], v[:], AF.Relu,
                                     scale=dinv_t[:, t:t + 1])
            else:
                u = upool.tile([128, DH], f32, tag="u")
                nc.scalar.activation(u[:], z1[:], AF.Copy,
                                     scale=dinv_t[:, t:t + 1])
                h2 = hpool.tile([128, DH], bf16, tag="h2")
                nc.scalar.activation(h2[:], u[:], AF.Relu,
                                     scale=dinv_t[:, t:t + 1])
            r = rows_of(t)
            nc.sync.dma_start(tab2_shard[t * 128:t * 128 + r, :], h2[0:r, :])

        nc.gpsimd.collective_compute(
            "AllGather", OP.bypass, replica_groups=groups,
            ins=[tab2_shard[:].opt()], outs=[tab2_full[:].opt()])

        # ---- phase 3: layer 2 aggregate + transform -> output ----
        for t in range(NT):
            agg = aggregate(t, tab2_full, DH)
            cp = cppool.tile([DH, 128], bf16, tag="cp")
            nc.scalar.activation(cp[:], agg[:], AF.Copy)
            z2 = psB.tile([128, DOUT], f32, tag="z2")
            nc.tensor.matmul(out=z2[:], lhsT=cp[:], rhs=W2b[:],
                             start=True, stop=True)
            u2 = upool.tile([128, DOUT], f32, tag="u2")
            nc.scalar.activation(u2[:], z2[:], AF.Copy,
                                 scale=dinv_t[:, t:t + 1])
            if b2_nz:
                v2 = upool.tile([128, DOUT], f32, tag="v2")
                nc.vector.tensor_tensor(v2[:], u2[:], b2r[:], op=OP.add)
                fin = v2
            else:
                fin = u2
            r = rows_of(t)
            nc.sync.dma_start(out_d[t * 128:t * 128 + r, :], fin[0:r, :])

    nc.compile()
    return nc


def kernel(x, edge_index, W1, b1, W2, b2):
    from concourse.bass_utils import run_bass_kernel_spmd

    x = np.asarray(x, dtype=np.float32)
    W1 = np.asarray(W1, dtype=np.float32)
    W2 = np.asarray(W2, dtype=np.float32)
    b1 = np.asarray(b1, dtype=np.float32)
    b2 = np.asarray(b2, dtype=np.float32)
    ei = np.asarray(edge_index)

    dinv, idx_maps, dl_maps, sched, TOT = _preprocess(ei)

    b1_nz = bool(np.any(b1 != 0))
    b2_nz = bool(np.any(b2 != 0))
    key = ("graph", TOT, tuple(tuple(w) for ws in sched for w in ws),
           b1_nz, b2_nz)
    if key not in _CACHE:
        _CACHE.clear()
        _CACHE[key] = _build(sched, TOT, b1_nz, b2_nz)
    nc = _CACHE[key]

    b1r = np.broadcast_to(b1.reshape(1, DH), (128, DH)).copy()
    b2r = np.broadcast_to(b2.reshape(1, DOUT), (128, DOUT)).copy()

    in_maps = []
    for c in range(NCORES):
        lo, hi = c * NPC, (c + 1) * NPC
        xs = np.zeros((NT * 128, DIN), dtype=np.float32)
        xs[:NPC] = x[lo:hi]
        dv = np.zeros((128, NT), dtype=np.float32)
        dvflat = np.zeros(NT * 128, dtype=np.float32)
        dvflat[:NPC] = dinv[lo:hi]
        dv[:] = dvflat.reshape(NT, 128).T
        in_maps.append({
            "x": xs, "dinv": dv,
            "idx": idx_maps[c], "dstloc": dl_maps[c],
            "W1": W1, "W2": W2, "b1": b1r, "b2": b2r,
        })

    res = run_bass_kernel_spmd(nc, in_maps, list(range(NCORES)))
    out = np.concatenate([res.results[c]["out"] for c in range(NCORES)], axis=0)
    return out.astype(np.float32)



# revision 3
# speedup vs baseline: 1.1980x; 1.1980x over previous
"""2-layer GCN encoder on 8 Trainium2 NeuronCores (Bass/Tile).

Algorithm (per layer, using GCNConv linearity: A_hat @ (x @ W) == (A_hat @ x) @ W):
  dinv = 1/sqrt(deg+1);  htab = bf16(dinv * x_layer_input)     (node-sharded)
  AllGather htab -> full gather table in DRAM
  per dst-node tile of 128: dma_gather the htab rows of all in-edges (plus a
  self-loop edge per node), reduce into PSUM via one-hot matmuls
  (aggT[feat,dst] += msg_chunk^T @ onehot(dst_local)), then
  z = aggT^T @ W (natural layout via lhsT=aggT), post-scale by dinv, bias, relu.

Host side does only index preprocessing (edge partitioning/sorting/padding)
and sharding; all float math runs on device.
"""

import math
import numpy as np
import ml_dtypes
from contextlib import ExitStack

# ---- static problem config (hardcoded per contract) ----
N = 100000
E = 1600000
DIN = 128
DH = 128
DOUT = 64
NCORES = 8
NPC = N // NCORES            # 12500 nodes per core
NT = math.ceil(NPC / 128)    # 98 dst tiles per core
LAST_ROWS = NPC - (NT - 1) * 128   # 84
WIN = 32768                  # int16 index window for dma_gather
NW = math.ceil(N / WIN)      # 4 windows
SENTINEL = 200.0             # dst_local value for pad slots (matches no iota lane)

_CACHE = {}


def _preprocess(edge_index):
    """Partition/sort/pad edges. Returns per-core index arrays + shared schedule."""
    src = np.ascontiguousarray(edge_index[0]).astype(np.int64)
    dst = np.ascontiguousarray(edge_index[1]).astype(np.int64)

    deg = np.bincount(dst, minlength=N).astype(np.float64) + 1.0
    dinv = (1.0 / np.sqrt(deg)).astype(np.float32)

    per_core = []
    counts = np.zeros((NCORES, NT * NW), dtype=np.int64)
    for c in range(NCORES):
        lo, hi = c * NPC, (c + 1) * NPC
        sel = (dst >= lo) & (dst < hi)
        es = np.concatenate([src[sel], np.arange(lo, hi, dtype=np.int64)])
        ed = np.concatenate([dst[sel] - lo, np.arange(NPC, dtype=np.int64)])
        t = ed >> 7
        w = es // WIN
        gid = t * NW + w
        order = np.argsort(gid, kind="stable")
        es, ed, gid = es[order], ed[order], gid[order]
        counts[c] = np.bincount(gid, minlength=NT * NW)
        per_core.append((es, ed, gid))

    cnt_max = counts.max(axis=0)
    cnt_pad = ((cnt_max + 127) // 128) * 128          # 0 stays 0
    slot_off = np.zeros(NT * NW, dtype=np.int64)
    slot_off[1:] = np.cumsum(cnt_pad)[:-1]
    TOT = int(cnt_pad.sum())

    # shared gather schedule: per tile, list of (window, n_chunks, slot_off)
    sched = []
    for t in range(NT):
        ws = []
        for w in range(NW):
            g = t * NW + w
            if cnt_pad[g] > 0:
                ws.append((w, int(cnt_pad[g] // 128), int(slot_off[g])))
        sched.append(ws)

    idx_maps, dl_maps = [], []
    for c in range(NCORES):
        es, ed, gid = per_core[c]
        cstart = np.zeros(NT * NW, dtype=np.int64)
        cstart[1:] = np.cumsum(counts[c])[:-1]
        rank = np.arange(len(es)) - cstart[gid]
        slot = slot_off[gid] + rank
        idx = np.zeros(TOT, dtype=np.int16)
        dl = np.full(TOT, SENTINEL, dtype=np.float32)
        idx[slot] = (es - (es // WIN) * WIN).astype(np.int16)
        dl[slot] = (ed & 127).astype(np.float32)
        # SBUF layouts: idx wrapped over 16 partitions (replicated x8),
        # dstloc wrapped over 128 partitions, one column per 128-edge chunk.
        idx_sb = np.tile(np.ascontiguousarray(idx.reshape(-1, 16).T), (8, 1))
        dl_sb = np.ascontiguousarray(dl.reshape(-1, 128).T).astype(ml_dtypes.bfloat16)
        idx_maps.append(idx_sb)
        dl_maps.append(dl_sb)

    return dinv, idx_maps, dl_maps, sched, TOT


def _build(sched, TOT, b1_nz, b2_nz):
    import concourse.bass as bass
    import concourse.tile as tile
    from concourse import bacc, mybir

    f32 = mybir.dt.float32
    bf16 = mybir.dt.bfloat16
    AF = mybir.ActivationFunctionType
    OP = mybir.AluOpType

    nc = bacc.Bacc("TRN2", target_bir_lowering=False, debug=False,
                   num_devices=NCORES, num_swdge_queues=4)

    x_d = nc.dram_tensor("x", [NT * 128, DIN], f32, kind="ExternalInput").ap()
    dinv_d = nc.dram_tensor("dinv", [128, NT], f32, kind="ExternalInput").ap()
    idx_d = nc.dram_tensor("idx", [128, TOT // 16], mybir.dt.int16,
                           kind="ExternalInput").ap()
    dl_d = nc.dram_tensor("dstloc", [128, TOT // 128], bf16,
                          kind="ExternalInput").ap()
    W1_d = nc.dram_tensor("W1", [DIN, DH], f32, kind="ExternalInput").ap()
    W2_d = nc.dram_tensor("W2", [DH, DOUT], f32, kind="ExternalInput").ap()
    b1_d = nc.dram_tensor("b1", [128, DH], f32, kind="ExternalInput").ap()
    b2_d = nc.dram_tensor("b2", [128, DOUT], f32, kind="ExternalInput").ap()
    out_d = nc.dram_tensor("out", [NPC, DOUT], f32, kind="ExternalOutput").ap()

    groups = [list(range(NCORES))]

    with tile.TileContext(nc) as tc, ExitStack() as ctx:
        dram = ctx.enter_context(tc.tile_pool(name="dram", bufs=1, space="DRAM"))
        tab1_shard = dram.tile([NPC, DIN], bf16)
        tab1_full = dram.tile([N, DIN], bf16, addr_space="Shared")
        tab2_shard = dram.tile([NPC, DH], bf16)
        tab2_full = dram.tile([N, DH], bf16, addr_space="Shared")

        const = ctx.enter_context(tc.tile_pool(name="const", bufs=1))
        xpool = ctx.enter_context(tc.tile_pool(name="xp", bufs=3))
        hpool = ctx.enter_context(tc.tile_pool(name="hp", bufs=3))
        msgpool = ctx.enter_context(tc.tile_pool(name="msg", bufs=3))
        mpool = ctx.enter_context(tc.tile_pool(name="mm", bufs=3))
        cppool = ctx.enter_context(tc.tile_pool(name="cp", bufs=3))
        upool = ctx.enter_context(tc.tile_pool(name="up", bufs=3))
        psA = ctx.enter_context(tc.tile_pool(name="psA", bufs=2, space="PSUM"))
        psB = ctx.enter_context(tc.tile_pool(name="psB", bufs=2, space="PSUM"))

        # ---- constants ----
        iota_i = const.tile([128, 128], mybir.dt.int32)
        nc.gpsimd.iota(iota_i[:], pattern=[[1, 128]], base=0, channel_multiplier=0)
        iota_b = const.tile([128, 128], bf16)
        nc.vector.tensor_copy(iota_b[:], iota_i[:])

        dinv_t = const.tile([128, NT], f32)
        nc.sync.dma_start(dinv_t[:], dinv_d[:])
        idx_t = const.tile([128, TOT // 16], mybir.dt.int16)
        nc.sync.dma_start(idx_t[:], idx_d[:])
        dl_t = const.tile([128, TOT // 128], bf16)
        nc.sync.dma_start(dl_t[:], dl_d[:])

        W1f = const.tile([DIN, DH], f32)
        nc.sync.dma_start(W1f[:], W1_d[:])
        W1b = const.tile([DIN, DH], bf16)
        nc.vector.tensor_copy(W1b[:], W1f[:])
        W2f = const.tile([DH, DOUT], f32)
        nc.sync.dma_start(W2f[:], W2_d[:])
        W2b = const.tile([DH, DOUT], bf16)
        nc.vector.tensor_copy(W2b[:], W2f[:])
        if b1_nz:
            b1r = const.tile([128, DH], f32)
            nc.sync.dma_start(b1r[:], b1_d[:])
        if b2_nz:
            b2r = const.tile([128, DOUT], f32)
            nc.sync.dma_start(b2r[:], b2_d[:])

        def rows_of(t):
            return LAST_ROWS if t == NT - 1 else 128

        # ---- phase 1: layer-1 gather table (h1 = dinv * x, bf16) ----
        for t in range(NT):
            xt = xpool.tile([128, DIN], f32, tag="xt")
            nc.sync.dma_start(xt[:], x_d[t * 128:(t + 1) * 128, :])
            h1 = hpool.tile([128, DIN], bf16, tag="h1")
            nc.scalar.activation(h1[:], xt[:], AF.Copy, scale=dinv_t[:, t:t + 1])
            r = rows_of(t)
            nc.sync.dma_start(tab1_shard[t * 128:t * 128 + r, :], h1[0:r, :])

        nc.gpsimd.collective_compute(
            "AllGather", OP.bypass, replica_groups=groups,
            ins=[tab1_shard[:].opt()], outs=[tab1_full[:].opt()])

        gq = [0]  # round-robin SWDGE queue assignment across gathers

        def aggregate(t, tab_full, dim):
            """Gather in-edge rows for dst tile t and reduce into PSUM.
            Returns aggT PSUM tile [dim(feat), 128(dst)]."""
            ws = sched[t]
            CH = sum(nch for (_, nch, _) in ws)
            msg = msgpool.tile([128, CH, dim], bf16, tag="msg")
            cum = 0
            for (w, nch, soff) in ws:
                wrows = min(WIN, N - w * WIN)
                nc.gpsimd.dma_gather(
                    msg[:, cum:cum + nch, :],
                    tab_full[w * WIN:w * WIN + wrows, :],
                    idx_t[:, soff // 16: soff // 16 + nch * 8],
                    num_idxs=nch * 128,
                    num_idxs_reg=nch * 128,
                    elem_size=dim,
                    queue_num=gq[0],
                )
                gq[0] = (gq[0] + 1) % 4
                cum += nch
            # one-hot M: [128(edge), CH*128(dst-lane)]
            cb = None
            for (w, nch, soff) in ws:
                if cb is None:
                    cb = soff // 128
            M = mpool.tile([128, CH * 128], bf16, tag="M")
            m_ap = M[:]
            out3 = bass.AP(m_ap.tensor, m_ap.offset,
                           [list(m_ap.ap[0]), [128, CH], [1, 128]])
            in0 = dl_t[:, cb:cb + CH].to_broadcast([128, CH, 128])
            io_ap = iota_b[:]
            in1 = bass.AP(io_ap.tensor, io_ap.offset,
                          [list(io_ap.ap[0]), [0, CH], [1, 128]])
            nc.vector.tensor_tensor(out3, in0, in1, op=OP.is_equal)

            agg = psA.tile([dim, 128], f32, tag="agg")
            for k in range(CH):
                nc.tensor.matmul(
                    out=agg[:],
                    lhsT=msg[:, k:k + 1, :].opt(),
                    rhs=M[:, k * 128:(k + 1) * 128],
                    start=(k == 0), stop=(k == CH - 1))
            return agg

        # ---- phase 2: layer 1 aggregate + transform -> layer-2 table ----
        for t in range(NT):
            agg = aggregate(t, tab1_full, DIN)
            cp = cppool.tile([DIN, 128], bf16, tag="cp")
            nc.scalar.activation(cp[:], agg[:], AF.Copy)
            z1 = psB.tile([128, DH], f32, tag="z1")
            nc.tensor.matmul(out=z1[:], lhsT=cp[:], rhs=W1b[:],
                             start=True, stop=True)
            # h2 = dinv * relu(dinv * z1 + b1)  (dinv per-partition here)
            if b1_nz:
                u = upool.tile([128, DH], f32, tag="u")
                nc.scalar.activation(u[:], z1[:], AF.Copy,
                                     scale=dinv_t[:, t:t + 1])
                v = upool.tile([128, DH], f32, tag="v")
                nc.vector.tensor_tensor(v[:], u[:], b1r[:], op=OP.add)
                h2 = hpool.tile([128, DH], bf16, tag="h2")
                nc.scalar.activation(h2[:], v[:], AF.Relu,
                                     scale=dinv_t[:, t:t + 1])
            else:
                u = upool.tile([128, DH], f32, tag="u")
                nc.scalar.activation(u[:], z1[:], AF.Copy,
                                     scale=dinv_t[:, t:t + 1])
                h2 = hpool.tile([128, DH], bf16, tag="h2")
                nc.scalar.activation(h2[:], u[:], AF.Relu,
                                     scale=dinv_t[:, t:t + 1])
            r = rows_of(t)
            nc.sync.dma_start(tab2_shard[t * 128:t * 128 + r, :], h2[0:r, :])

        nc.gpsimd.collective_compute(
            "AllGather", OP.bypass, replica_groups=groups,
            ins=[tab2_shard[:].opt()], outs=[tab2_full[:].opt()])

        # ---- phase 3: layer 2 aggregate + transform -> output ----
        for t in range(NT):
            agg = aggregate(t, tab2_full, DH)
            cp = cppool.tile([DH, 128], bf16, tag="cp")
            nc.scalar.activation(cp[:], agg[:], AF.Copy)
            z2 = psB.tile([128, DOUT], f32, tag="z2")
            nc.tensor.matmul(out=z2[:], lhsT=cp[:], rhs=W2b[:],
                             start=True, stop=True)
            u2 = upool.tile([128, DOUT], f32, tag="u2")
            nc.scalar.activation(u2[:], z2[:], AF.Copy,
                                 scale=dinv_t[:, t:t + 1])
            if b2_nz:
                v2 = upool.tile([128, DOUT], f32, tag="v2")
                nc.vector.tensor_tensor(v2[:], u2[:], b2r[:], op=OP.add)
                fin = v2
            else:
                fin = u2
            r = rows_of(t)
            nc.sync.dma_start(out_d[t * 128:t * 128 + r, :], fin[0:r, :])

    nc.compile()
    return nc


def kernel(x, edge_index, W1, b1, W2, b2):
    from concourse.bass_utils import run_bass_kernel_spmd

    x = np.asarray(x, dtype=np.float32)
    W1 = np.asarray(W1, dtype=np.float32)
    W2 = np.asarray(W2, dtype=np.float32)
    b1 = np.asarray(b1, dtype=np.float32)
    b2 = np.asarray(b2, dtype=np.float32)
    ei = np.asarray(edge_index)

    dinv, idx_maps, dl_maps, sched, TOT = _preprocess(ei)

    b1_nz = bool(np.any(b1 != 0))
    b2_nz = bool(np.any(b2 != 0))
    key = ("graph", TOT, tuple(tuple(w) for ws in sched for w in ws),
           b1_nz, b2_nz)
    if key not in _CACHE:
        _CACHE.clear()
        _CACHE[key] = _build(sched, TOT, b1_nz, b2_nz)
    nc = _CACHE[key]

    b1r = np.broadcast_to(b1.reshape(1, DH), (128, DH)).copy()
    b2r = np.broadcast_to(b2.reshape(1, DOUT), (128, DOUT)).copy()

    in_maps = []
    for c in range(NCORES):
        lo, hi = c * NPC, (c + 1) * NPC
        xs = np.zeros((NT * 128, DIN), dtype=np.float32)
        xs[:NPC] = x[lo:hi]
        dv = np.zeros((128, NT), dtype=np.float32)
        dvflat = np.zeros(NT * 128, dtype=np.float32)
        dvflat[:NPC] = dinv[lo:hi]
        dv[:] = dvflat.reshape(NT, 128).T
        in_maps.append({
            "x": xs, "dinv": dv,
            "idx": idx_maps[c], "dstloc": dl_maps[c],
            "W1": W1, "W2": W2, "b1": b1r, "b2": b2r,
        })

    res = run_bass_kernel_spmd(nc, in_maps, list(range(NCORES)))
    out = np.concatenate([res.results[c]["out"] for c in range(NCORES)], axis=0)
    return out.astype(np.float32)



# revision 6
# speedup vs baseline: 1.4864x; 1.2408x over previous
"""2-layer GCN encoder on 8 Trainium2 NeuronCores (Bass/Tile).

v2: SBUF-resident gather table. The per-edge gather of h[src] is the
bottleneck; random 256B HBM reads run at ~30-80 GB/s effective. Instead the
(bf16) gather table is staged into SBUF one 32768-node window at a time
(sequential HBM loads at line rate) and edges are gathered SBUF->SBUF with
dma_gather(transpose=True), spread across all 4 SWDGE queues.

The transposed gather output msgT[feat, edge] feeds a fused per-chunk
pipeline on TensorE (GCNConv linearity):
  V[e, o]   = sum_f msgT[f, e] * W[f, o]      (transform per edge)
  z[dst, o] += sum_e M[e, dst] * V[e, o]      (one-hot scatter per dst tile)
with z accumulated across windows in an SBUF buffer per dst tile.

Host side does only index preprocessing (edge partitioning/sorting/padding)
and sharding; all float math runs on device.
"""

import math
import numpy as np
import ml_dtypes
from contextlib import ExitStack

# ---- static problem config (hardcoded per contract) ----
N = 100000
E = 1600000
DIN = 128
DH = 128
DOUT = 64
NCORES = 8
NPC = N // NCORES            # 12500 nodes per core
NT = math.ceil(NPC / 128)    # 98 dst tiles per core
LAST_ROWS = NPC - (NT - 1) * 128   # 84
WIN = 32768                  # int16 index window
NW = math.ceil(N / WIN)      # 4 windows
SENTINEL = 200.0             # dst_local value for pad slots (matches no iota lane)

_CACHE = {}


def _preprocess(edge_index):
    """Partition/sort/pad edges. Returns per-core index arrays + shared schedule."""
    src = np.ascontiguousarray(edge_index[0]).astype(np.int64)
    dst = np.ascontiguousarray(edge_index[1]).astype(np.int64)

    deg = np.bincount(dst, minlength=N).astype(np.float64) + 1.0
    dinv = (1.0 / np.sqrt(deg)).astype(np.float32)

    per_core = []
    counts = np.zeros((NCORES, NT * NW), dtype=np.int64)
    for c in range(NCORES):
        lo, hi = c * NPC, (c + 1) * NPC
        sel = (dst >= lo) & (dst < hi)
        es = np.concatenate([src[sel], np.arange(lo, hi, dtype=np.int64)])
        ed = np.concatenate([dst[sel] - lo, np.arange(NPC, dtype=np.int64)])
        t = ed >> 7
        w = es // WIN
        gid = t * NW + w
        order = np.argsort(gid, kind="stable")
        es, ed, gid = es[order], ed[order], gid[order]
        counts[c] = np.bincount(gid, minlength=NT * NW)
        per_core.append((es, ed, gid))

    cnt_max = counts.max(axis=0)
    cnt_pad = ((cnt_max + 127) // 128) * 128          # 0 stays 0
    slot_off = np.zeros(NT * NW, dtype=np.int64)
    slot_off[1:] = np.cumsum(cnt_pad)[:-1]
    TOT = int(cnt_pad.sum())

    # window-major schedule: per window, list of (tile, n_chunks, slot_off)
    sched_w = []
    for w in range(NW):
        ts = []
        for t in range(NT):
            g = t * NW + w
            if cnt_pad[g] > 0:
                ts.append((t, int(cnt_pad[g] // 128), int(slot_off[g])))
        sched_w.append(ts)
    # first window with edges per tile (for copy-vs-accumulate in zbuf)
    first_w = [min(w for w in range(NW) if cnt_pad[t * NW + w] > 0)
               for t in range(NT)]

    idx_maps, dl_maps = [], []
    for c in range(NCORES):
        es, ed, gid = per_core[c]
        cstart = np.zeros(NT * NW, dtype=np.int64)
        cstart[1:] = np.cumsum(counts[c])[:-1]
        rank = np.arange(len(es)) - cstart[gid]
        slot = slot_off[gid] + rank
        idx = np.zeros(TOT, dtype=np.int16)
        dl = np.full(TOT, SENTINEL, dtype=np.float32)
        # SBUF window layout: node wn (within window) lives at partition
        # wn>>8, byte offset (wn&255)*256 -> gather token/rank index is the
        # bit-rotation ((wn&255)<<7) | (wn>>8).
        wn = es - (es // WIN) * WIN
        idx[slot] = (((wn & 255) << 7) | (wn >> 8)).astype(np.int16)
        dl[slot] = (ed & 127).astype(np.float32)
        # SBUF layouts: idx wrapped over 16 partitions (replicated x8),
        # dstloc wrapped over 128 partitions, one column per 128-edge chunk.
        idx_sb = np.tile(np.ascontiguousarray(idx.reshape(-1, 16).T), (8, 1))
        dl_sb = np.ascontiguousarray(dl.reshape(-1, 128).T).astype(ml_dtypes.bfloat16)
        idx_maps.append(idx_sb)
        dl_maps.append(dl_sb)

    return dinv, idx_maps, dl_maps, sched_w, first_w, TOT


def _build(sched_w, first_w, TOT, b1_nz, b2_nz):
    import concourse.bass as bass
    import concourse.tile as tile
    from concourse import bacc, mybir

    f32 = mybir.dt.float32
    bf16 = mybir.dt.bfloat16
    AF = mybir.ActivationFunctionType
    OP = mybir.AluOpType

    nc = bacc.Bacc("TRN2", target_bir_lowering=False, debug=False,
                   num_devices=NCORES, num_swdge_queues=4)

    x_d = nc.dram_tensor("x", [NT * 128, DIN], f32, kind="ExternalInput").ap()
    dinv_d = nc.dram_tensor("dinv", [128, NT], f32, kind="ExternalInput").ap()
    idx_d = nc.dram_tensor("idx", [128, TOT // 16], mybir.dt.int16,
                           kind="ExternalInput").ap()
    dl_d = nc.dram_tensor("dstloc", [128, TOT // 128], bf16,
                          kind="ExternalInput").ap()
    W1_d = nc.dram_tensor("W1", [DIN, DH], f32, kind="ExternalInput").ap()
    W2_d = nc.dram_tensor("W2", [DH, DOUT], f32, kind="ExternalInput").ap()
    b1_d = nc.dram_tensor("b1", [128, DH], f32, kind="ExternalInput").ap()
    b2_d = nc.dram_tensor("b2", [128, DOUT], f32, kind="ExternalInput").ap()
    out_d = nc.dram_tensor("out", [NPC, DOUT], f32, kind="ExternalOutput").ap()

    groups = [list(range(NCORES))]

    with tile.TileContext(nc) as tc, ExitStack() as ctx:
        dram = ctx.enter_context(tc.tile_pool(name="dram", bufs=1, space="DRAM"))
        tab1_shard = dram.tile([NPC, DIN], bf16)
        tab1_full = dram.tile([N, DIN], bf16, addr_space="Shared")
        tab2_shard = dram.tile([NPC, DH], bf16)
        tab2_full = dram.tile([N, DH], bf16, addr_space="Shared")

        const = ctx.enter_context(tc.tile_pool(name="const", bufs=1))
        zpool = ctx.enter_context(tc.tile_pool(name="zp", bufs=1))
        winpool = ctx.enter_context(tc.tile_pool(name="win", bufs=1))
        xpool = ctx.enter_context(tc.tile_pool(name="xp", bufs=3))
        hpool = ctx.enter_context(tc.tile_pool(name="hp", bufs=3))
        msgpool = ctx.enter_context(tc.tile_pool(name="msg", bufs=3))
        mpool = ctx.enter_context(tc.tile_pool(name="mm", bufs=3))
        vpool = ctx.enter_context(tc.tile_pool(name="vp", bufs=3))
        upool = ctx.enter_context(tc.tile_pool(name="up", bufs=3))
        psV = ctx.enter_context(tc.tile_pool(name="psV", bufs=2, space="PSUM"))
        psZ = ctx.enter_context(tc.tile_pool(name="psZ", bufs=2, space="PSUM"))

        # ---- constants ----
        iota_i = const.tile([128, 128], mybir.dt.int32)
        nc.gpsimd.iota(iota_i[:], pattern=[[1, 128]], base=0, channel_multiplier=0)
        iota_b = const.tile([128, 128], bf16)
        nc.vector.tensor_copy(iota_b[:], iota_i[:])

        dinv_t = const.tile([128, NT], f32)
        nc.sync.dma_start(dinv_t[:], dinv_d[:])
        idx_t = const.tile([128, TOT // 16], mybir.dt.int16)
        nc.sync.dma_start(idx_t[:], idx_d[:])
        dl_t = const.tile([128, TOT // 128], bf16)
        nc.sync.dma_start(dl_t[:], dl_d[:])

        W1f = const.tile([DIN, DH], f32)
        nc.sync.dma_start(W1f[:], W1_d[:])
        W1b = const.tile([DIN, DH], bf16)
        nc.vector.tensor_copy(W1b[:], W1f[:])
        W2f = const.tile([DH, DOUT], f32)
        nc.sync.dma_start(W2f[:], W2_d[:])
        W2b = const.tile([DH, DOUT], bf16)
        nc.vector.tensor_copy(W2b[:], W2f[:])
        if b1_nz:
            b1r = const.tile([128, DH], f32)
            nc.sync.dma_start(b1r[:], b1_d[:])
        if b2_nz:
            b2r = const.tile([128, DOUT], f32)
            nc.sync.dma_start(b2r[:], b2_d[:])

        def rows_of(t):
            return LAST_ROWS if t == NT - 1 else 128

        # ---- phase 1: layer-1 gather table (h1 = dinv * x, bf16) ----
        for t in range(NT):
            xt = xpool.tile([128, DIN], f32, tag="xt")
            nc.sync.dma_start(xt[:], x_d[t * 128:(t + 1) * 128, :])
            h1 = hpool.tile([128, DIN], bf16, tag="h1")
            nc.scalar.activation(h1[:], xt[:], AF.Copy, scale=dinv_t[:, t:t + 1])
            r = rows_of(t)
            nc.sync.dma_start(tab1_shard[t * 128:t * 128 + r, :], h1[0:r, :])

        nc.gpsimd.collective_compute(
            "AllGather", OP.bypass, replica_groups=groups,
            ins=[tab1_shard[:].opt()], outs=[tab1_full[:].opt()])

        gq = [0]  # round-robin SWDGE queue

        def load_window(tab_full, w):
            """Stage window w of the gather table into SBUF.
            Layout: win[p, r*128+f] = tab[w*WIN + p*256 + r, f]."""
            win = winpool.tile([128, WIN], bf16, tag="win")
            wrows = min(WIN, N - w * WIN)
            full_p = wrows // 256          # partitions fully populated
            rem = wrows - full_p * 256     # leftover rows on partition full_p
            src = tab_full[w * WIN:w * WIN + wrows, :]
            t3 = src.tensor
            base = src.offset
            if full_p > 0:
                in_ap = bass.AP(t3, base,
                                [[256 * 128, full_p], [128, 256], [1, 128]])
                out_ap = win[0:full_p, :]
                o3 = bass.AP(out_ap.tensor, out_ap.offset,
                             [list(out_ap.ap[0]), [128, 256], [1, 128]])
                nc.sync.dma_start(o3, in_ap)
            if rem > 0:
                in_ap = bass.AP(t3, base + full_p * 256 * 128,
                                [[0, 1], [128, rem], [1, 128]])
                out_ap = win[full_p:full_p + 1, 0:rem * 128]
                o3 = bass.AP(out_ap.tensor, out_ap.offset,
                             [list(out_ap.ap[0]), [128, rem], [1, 128]])
                nc.sync.dma_start(o3, in_ap)
            return win

        def layer(tab_full, Wb, dim_out):
            """One GCN layer aggregation+transform; returns zbuf SBUF tile
            [128(dst), NT*dim_out] bf16 (pre dst-side dinv scale / bias)."""
            zbuf = zpool.tile([128, NT * dim_out], bf16, tag="zbuf")
            for w in range(NW):
                win = load_window(tab_full, w)
                for (t, nch, soff) in sched_w[w]:
                    S = nch * 128
                    msgT = msgpool.tile([128, 1, S], bf16, tag="msgT")
                    nc.gpsimd.dma_gather(
                        msgT[:],
                        win[:],
                        idx_t[:, soff // 16: soff // 16 + nch * 8],
                        num_idxs=S,
                        num_idxs_reg=S,
                        elem_size=128,
                        transpose=True,
                        sbuf_tokens_per_rank=128,
                        sbuf_free_dim_per_rank=256,
                        sbuf_free_dim_pad_per_rank=0,
                        sbuf_byte_offset=0,
                        queue_num=gq[0],
                    )
                    gq[0] = (gq[0] + 1) % 4
                    # one-hot M: [128(edge-slot), nch*128(dst-lane)]
                    cb = soff // 128
                    M = mpool.tile([128, nch * 128], bf16, tag="M")
                    m_ap = M[:]
                    out3 = bass.AP(m_ap.tensor, m_ap.offset,
                                   [list(m_ap.ap[0]), [128, nch], [1, 128]])
                    in0 = dl_t[:, cb:cb + nch].to_broadcast([128, nch, 128])
                    io_ap = iota_b[:]
                    in1 = bass.AP(io_ap.tensor, io_ap.offset,
                                  [list(io_ap.ap[0]), [0, nch], [1, 128]])
                    nc.vector.tensor_tensor(out3, in0, in1, op=OP.is_equal)

                    z = psZ.tile([128, dim_out], f32, tag="z")
                    nq = (nch + 3) // 4
                    k = 0
                    for q in range(nq):
                        kn = min(4, nch - q * 4)
                        pv = psV.tile([128, 4, dim_out], f32, tag="pv")
                        for j in range(kn):
                            c = q * 4 + j
                            nc.tensor.matmul(
                                out=pv[:, j:j + 1, :].opt(),
                                lhsT=msgT[:, 0:1, c * 128:(c + 1) * 128].opt(),
                                rhs=Wb[:],
                                start=True, stop=True)
                        vb = vpool.tile([128, 4, dim_out], bf16, tag="vb")
                        nc.scalar.activation(
                            vb[:, 0:kn, :].opt(), pv[:, 0:kn, :].opt(), AF.Copy)
                        for j in range(kn):
                            c = q * 4 + j
                            nc.tensor.matmul(
                                out=z[:],
                                lhsT=M[:, c * 128:(c + 1) * 128],
                                rhs=vb[:, j:j + 1, :].opt(),
                                start=(c == 0), stop=(c == nch - 1))
                    zs = zbuf[:, t * dim_out:(t + 1) * dim_out]
                    if w == first_w[t]:
                        nc.vector.tensor_copy(zs, z[:])
                    else:
                        nc.vector.tensor_tensor(zs, zs, z[:], op=OP.add)
            return zbuf

        # ---- layer 1 ----
        zbuf = layer(tab1_full, W1b, DH)
        for t in range(NT):
            zs = zbuf[:, t * DH:(t + 1) * DH]
            # h2 = dinv * relu(dinv * z1 + b1)
            if b1_nz:
                u = upool.tile([128, DH], f32, tag="u")
                nc.scalar.activation(u[:], zs, AF.Copy,
                                     scale=dinv_t[:, t:t + 1])
                v = upool.tile([128, DH], f32, tag="v")
                nc.vector.tensor_tensor(v[:], u[:], b1r[:], op=OP.add)
                h2 = hpool.tile([128, DH], bf16, tag="h2")
                nc.scalar.activation(h2[:], v[:], AF.Relu,
                                     scale=dinv_t[:, t:t + 1])
            else:
                u = upool.tile([128, DH], f32, tag="u")
                nc.scalar.activation(u[:], zs, AF.Copy,
                                     scale=dinv_t[:, t:t + 1])
                h2 = hpool.tile([128, DH], bf16, tag="h2")
                nc.scalar.activation(h2[:], u[:], AF.Relu,
                                     scale=dinv_t[:, t:t + 1])
            r = rows_of(t)
            nc.sync.dma_start(tab2_shard[t * 128:t * 128 + r, :], h2[0:r, :])

        nc.gpsimd.collective_compute(
            "AllGather", OP.bypass, replica_groups=groups,
            ins=[tab2_shard[:].opt()], outs=[tab2_full[:].opt()])

        # ---- layer 2 ----
        zbuf2 = layer(tab2_full, W2b, DOUT)
        for t in range(NT):
            zs = zbuf2[:, t * DOUT:(t + 1) * DOUT]
            u2 = upool.tile([128, DOUT], f32, tag="u2")
            nc.scalar.activation(u2[:], zs, AF.Copy,
                                 scale=dinv_t[:, t:t + 1])
            if b2_nz:
                v2 = upool.tile([128, DOUT], f32, tag="v2")
                nc.vector.tensor_tensor(v2[:], u2[:], b2r[:], op=OP.add)
                fin = v2
            else:
                fin = u2
            r = rows_of(t)
            nc.sync.dma_start(out_d[t * 128:t * 128 + r, :], fin[0:r, :])

    nc.compile()
    return nc


def kernel(x, edge_index, W1, b1, W2, b2):
    from concourse.bass_utils import run_bass_kernel_spmd

    x = np.asarray(x, dtype=np.float32)
    W1 = np.asarray(W1, dtype=np.float32)
    W2 = np.asarray(W2, dtype=np.float32)
    b1 = np.asarray(b1, dtype=np.float32)
    b2 = np.asarray(b2, dtype=np.float32)
    ei = np.asarray(edge_index)

    dinv, idx_maps, dl_maps, sched_w, first_w, TOT = _preprocess(ei)

    b1_nz = bool(np.any(b1 != 0))
    b2_nz = bool(np.any(b2 != 0))
    key = ("graph", TOT, tuple(tuple(w) for ws in sched_w for w in ws),
           b1_nz, b2_nz)
    if key not in _CACHE:
        _CACHE.clear()
        _CACHE[key] = _build(sched_w, first_w, TOT, b1_nz, b2_nz)
    nc = _CACHE[key]

    b1r = np.broadcast_to(b1.reshape(1, DH), (128, DH)).copy()
    b2r = np.broadcast_to(b2.reshape(1, DOUT), (128, DOUT)).copy()

    in_maps = []
    for c in range(NCORES):
        lo, hi = c * NPC, (c + 1) * NPC
        xs = np.zeros((NT * 128, DIN), dtype=np.float32)
        xs[:NPC] = x[lo:hi]
        dv = np.zeros((128, NT), dtype=np.float32)
        dvflat = np.zeros(NT * 128, dtype=np.float32)
        dvflat[:NPC] = dinv[lo:hi]
        dv[:] = dvflat.reshape(NT, 128).T
        in_maps.append({
            "x": xs, "dinv": dv,
            "idx": idx_maps[c], "dstloc": dl_maps[c],
            "W1": W1, "W2": W2, "b1": b1r, "b2": b2r,
        })

    res = run_bass_kernel_spmd(nc, in_maps, list(range(NCORES)))
    out = np.concatenate([res.results[c]["out"] for c in range(NCORES)], axis=0)
    return out.astype(np.float32)
